# revision 9
# baseline (speedup 1.0000x reference)
"""Trainium2 Bass kernel for CreatorXSimGCL-style GNN message passing.

  mixed_item = item_emb + author_emb[item2author]
  ego = concat([user_emb, mixed_item])            # [N, 64], N = 300k
  acc = 0
  for k in 3 layers:
      ego = segment_sum(ego[col] * vals[:,None], row, N)   # COO SpMM
      ego += sign(ego) * l2_normalize(noise[k]) * 0.2
      acc += ego
  return (acc/3)[:200k], (acc/3)[200k:]

Distribution (8 cores): rows are degree-sorted into 128-row tiles assigned
round-robin to cores, so each core owns ALL edges of 1/8 of the output rows
(balanced); its partial segment_sum is final for those rows - only a
per-layer AllGather of post-epilogue row shards is needed (no all-reduce).

SpMM mechanism: edges are laid out host-side in "identity slot grids": for
each destination tile, slot (j, p) holds an edge whose destination is local
row p.  A dma_gather whose index *position* i encodes (partition i%128 = p)
then lands every gathered source row in the partition of its destination
row, so the scatter-add reduces to one multiply + free-axis reductions on
the vector engines - no one-hot matrices.  dma_gather indices are int16, so
the source table is processed in 10 banks of 32768 rows; per-(window,bank)
slot grids are padded to the max per-row bank-degree (the price of int16).

All floating-point math runs on device; host preprocessing is integer
sorting/layout plus data movement.
"""

import hashlib
import sys

import numpy as np

for _p in ("/opt/trn_rl_repo", "/root/.axon_site/_ro/trn_rl_repo"):
    if _p not in sys.path:
        sys.path.append(_p)

P = 128
BANK = 32768


class Cfg:
    def __init__(self, num_users, num_items, num_authors, D, n_layers, eps,
                 n_cores=8, GT=5):
        self.num_users = num_users
        self.num_items = num_items
        self.num_authors = num_authors
        self.D = D
        self.n_layers = n_layers
        self.eps = eps
        self.n_cores = n_cores
        self.GT = GT  # tiles per window
        self.N = num_users + num_items
        grp = P * n_cores
        self.NP = ((self.N + grp - 1) // grp) * grp
        self.T_global = self.NP // P
        self.T = self.T_global // n_cores
        self.R = self.T * P
        self.NB = (self.NP + BANK - 1) // BANK  # source banks


FULL = Cfg(num_users=200000, num_items=100000, num_authors=20000,
           D=64, n_layers=3, eps=0.2)


def _wrap_idx(flat):
    """int16 idx list -> dma_gather wrapped layout [128, n/16]."""
    n = len(flat)
    assert n % 16 == 0
    blk = flat.reshape(-1, 16).T            # [16, n/16]
    return np.tile(blk, (8, 1)).astype(np.int16)


def preprocess(cfg, row, col, vals, item2author):
    N, NP, NC, T, GT, NB = cfg.N, cfg.NP, cfg.n_cores, cfg.T, cfg.GT, cfg.NB
    E = row.shape[0]

    deg = np.bincount(row, minlength=NP).astype(np.int64)
    order_rows = np.argsort(-deg, kind="stable")
    pos = np.empty(NP, np.int64)
    pos[order_rows] = np.arange(NP)
    gt_ = pos >> 7
    gp_ = pos & 127
    new_row = (gt_ % NC) * cfg.R + (gt_ // NC) * P + gp_   # [NP]

    # edge keys
    dpos = pos[row]                  # dest position in sorted order
    src = new_row[col]               # source row in ego_full layout
    bank = src >> 15

    # windows of GT local tiles
    n_win = (T + GT - 1) // GT
    win_sizes = [min(GT, T - w * GT) for w in range(n_win)]

    # per (dest row, bank) counts -> per (window, bank) capacity
    key_rb = dpos * NB + bank
    cnt_rb = np.bincount(key_rb, minlength=NP * NB).reshape(NP, NB)
    # capacity: max over all cores and all rows of the window's tiles
    # global tile t = 8*i + c ; local tile i = t//8 ; window w = i//GT
    gt_of_pos = np.arange(NP) >> 7
    li_of_pos = gt_of_pos // NC
    w_of_pos = li_of_pos // GT
    caps = np.zeros((n_win, NB), np.int64)
    for w in range(n_win):
        sel = w_of_pos == w
        caps[w] = cnt_rb[sel].max(axis=0)

    # slot offsets
    # per window: S_w = sum_b GT_w * caps[w,b] slots ; G layout (b, i, j)
    bankoff = []  # [n_win][NB] slot offset of bank block
    S_w = []
    for w in range(n_win):
        offs = np.concatenate([[0], np.cumsum(win_sizes[w] * caps[w])[:-1]])
        bankoff.append(offs)
        S_w.append(int((win_sizes[w] * caps[w]).sum()))
    S_tot = int(np.sum(S_w))
    s_woff = np.concatenate([[0], np.cumsum(S_w)[:-1]]).astype(np.int64)

    # occurrence index of each edge within its (dest row, bank) group
    ekey = dpos * NB + bank
    eorder = np.argsort(ekey, kind="stable")
    ek_s = ekey[eorder]
    diffs = np.diff(ek_s, prepend=-1) != 0
    run_id = np.cumsum(diffs) - 1
    run_starts = np.flatnonzero(diffs)
    occ = np.arange(E) - run_starts[run_id]

    # slot coordinates for every edge (in eorder order)
    dp_s = dpos[eorder]
    b_s = bank[eorder]
    src_s = src[eorder]
    v_s = vals[eorder]
    gt_s = dp_s >> 7
    p_s = dp_s & 127
    c_s = gt_s % NC
    li_s = gt_s // NC
    w_s = li_s // GT
    il_s = li_s - w_s * GT
    slot = s_woff[w_s] + bankoff_arr_lookup(bankoff, w_s, b_s) \
        + il_s * caps[w_s, b_s] + occ

    # value grid [NC, 128, S_tot] and idx grid
    valg = np.zeros((NC, P, S_tot), np.float32)
    valg[c_s, p_s, slot] = v_s
    idxg = np.zeros((NC, P, S_tot), np.int16)
    idxg[c_s, p_s, slot] = (src_s - b_s * BANK).astype(np.int16)

    # wrapped idx inputs per core: for gather (w,b): flat list over
    # (slot_local, p): position i = slot_local*128 + p
    idx_wrapped = np.zeros((NC, P, S_tot * 8), np.int16)
    for c in range(NC):
        flat = idxg[c].T.reshape(-1)  # [(slot, p)] slot-major
        idx_wrapped[c] = _wrap_idx(flat)

    # shard row bookkeeping (original ids per core in (li, p) order)
    shard_rows = order_rows.reshape(cfg.T_global, P)
    shard_orig = np.stack([
        shard_rows[np.arange(T) * NC + c].reshape(-1) for c in range(NC)])

    is_item = (shard_orig >= cfg.num_users) & (shard_orig < N)
    authidx = np.zeros((NC, cfg.R), np.int64)
    authidx[is_item] = item2author[
        (shard_orig[is_item] - cfg.num_users).astype(np.int64)]
    amask = is_item.astype(np.float32)
    # wrapped author idx per core per window: NI = nt*128, position
    # i = i_loc*128 + p -> author index of row (li=w*GT+i_loc, p)
    auth_wrapped = np.zeros((NC, P, T * 8), np.int16)
    for c in range(NC):
        flat = authidx[c].astype(np.int16)  # [(li, p)] already row-major
        auth_wrapped[c] = _wrap_idx(flat)

    return dict(order_rows=order_rows, new_row=new_row, caps=caps,
                bankoff=bankoff, S_w=S_w, s_woff=s_woff,
                win_sizes=win_sizes, n_win=n_win,
                valg=valg, idx_wrapped=idx_wrapped,
                shard_orig=shard_orig, amask=amask,
                auth_wrapped=auth_wrapped)


def bankoff_arr_lookup(bankoff, w_s, b_s):
    arr = np.stack(bankoff)  # [n_win, NB]
    return arr[w_s, b_s]


def build_in_maps(cfg, meta, user_emb, item_emb, author_emb, noise):
    NC, T, R, D = cfg.n_cores, cfg.T, cfg.R, cfg.D
    N = cfg.N
    base = np.zeros((cfg.NP, D), np.float32)
    base[:cfg.num_users] = user_emb
    base[cfg.num_users:N] = item_emb
    noise_ext = np.zeros((cfg.n_layers, cfg.NP, D), np.float32)
    noise_ext[:, :N] = noise

    in_maps = []
    for c in range(NC):
        so = meta["shard_orig"][c]
        ego0 = base[so]
        ego0_t = ego0.reshape(T, P, D).transpose(1, 0, 2).reshape(P, T * D)
        nz = noise_ext[:, so]
        nz_t = nz.reshape(cfg.n_layers, T, P, D).transpose(0, 2, 1, 3) \
                 .reshape(cfg.n_layers, P, T * D)
        in_maps.append({
            "idxg": np.ascontiguousarray(meta["idx_wrapped"][c]),
            "valg": np.ascontiguousarray(meta["valg"][c]),
            "ego0": np.ascontiguousarray(ego0_t),
            "authw": np.ascontiguousarray(meta["auth_wrapped"][c]),
            "amask": np.ascontiguousarray(
                meta["amask"][c].reshape(T, P).T),   # [P, T]
            "noisec": np.ascontiguousarray(nz_t),
            "authemb": np.ascontiguousarray(author_emb.astype(np.float32)),
        })
    return in_maps


def build_program(cfg, meta):
    from concourse import bacc, bass, mybir, tile

    f32 = mybir.dt.float32
    i16 = mybir.dt.int16
    D = cfg.D
    T, R, NP, NB, GT = cfg.T, cfg.R, cfg.NP, cfg.NB, cfg.GT
    L = cfg.n_layers
    caps = meta["caps"]
    n_win = meta["n_win"]
    win_sizes = meta["win_sizes"]
    S_w = meta["S_w"]
    s_woff = meta["s_woff"]
    inv_eps_sq = 1.0 / (cfg.eps * cfg.eps)
    sqrt_bias = inv_eps_sq * 1e-24

    nc = bacc.Bacc("TRN2", target_bir_lowering=False, debug=False,
                   num_devices=cfg.n_cores)

    S_tot = int(np.sum(S_w))
    idxg_t = nc.dram_tensor("idxg", [P, S_tot * 8], i16, kind="ExternalInput")
    valg_t = nc.dram_tensor("valg", [P, S_tot], f32, kind="ExternalInput")
    ego0_t = nc.dram_tensor("ego0", [P, T * D], f32, kind="ExternalInput")
    authw_t = nc.dram_tensor("authw", [P, T * 8], i16, kind="ExternalInput")
    amask_t = nc.dram_tensor("amask", [P, T], f32, kind="ExternalInput")
    noisec_t = nc.dram_tensor("noisec", [L, P, T * D], f32,
                              kind="ExternalInput")
    authemb_t = nc.dram_tensor("authemb", [cfg.num_authors, D], f32,
                               kind="ExternalInput")
    out_t = nc.dram_tensor("outshard", [R, D], f32, kind="ExternalOutput")

    rg = [list(range(cfg.n_cores))]

    with tile.TileContext(nc) as tc:
        with (
            tc.tile_pool(name="dram", bufs=1, space="DRAM") as dram,
            tc.tile_pool(name="persist", bufs=1) as persist,
            tc.tile_pool(name="io", bufs=2) as io,
            tc.tile_pool(name="gat", bufs=1) as gat,
            tc.tile_pool(name="work", bufs=2) as work,
        ):
            eshard = dram.tile([R, D], f32, name="eshard")
            accd = dram.tile([R, D], f32, name="accd")
            ego_full = [dram.tile([NP, D], f32, addr_space="Shared",
                                  name=f"egofull{i}") for i in range(L)]

            biast = persist.tile([P, 1], f32, tag="biast")
            nc.vector.memset(biast[:], sqrt_bias)

            def shard_rows_ap(dt_, i0, nt):
                return dt_[i0 * P:(i0 + nt) * P, :].rearrange(
                    "(t p) d -> p t d", p=P)

            # ---- init: author mixing, one dma_gather per window ----
            for w in range(n_win):
                i0 = w * GT
                nt = win_sizes[w]
                e0 = io.tile([P, nt * D], f32, tag="e0")
                nc.sync.dma_start(out=e0[:], in_=ego0_t[:, i0 * D:(i0 + nt) * D])
                ai = io.tile([P, nt * 8], i16, tag="ai")
                nc.sync.dma_start(out=ai[:], in_=authw_t[:, i0 * 8:(i0 + nt) * 8])
                mk = io.tile([P, nt], f32, tag="mk")
                nc.sync.dma_start(out=mk[:], in_=amask_t[:, i0:i0 + nt])
                ag = gat.tile([P, nt * D], f32, tag="AG")
                nc.gpsimd.dma_gather(
                    out_ap=ag[:].rearrange("p (t d) -> p t d", d=D),
                    in_ap=authemb_t[:], idxs_ap=ai[:],
                    num_idxs=nt * P, num_idxs_reg=nt * P, elem_size=D,
                    single_packet=False)
                nc.vector.tensor_tensor(
                    out=ag[:].rearrange("p (t d) -> p t d", d=D),
                    in0=ag[:].rearrange("p (t d) -> p t d", d=D),
                    in1=mk[:].unsqueeze(2).to_broadcast([P, nt, D]),
                    op=mybir.AluOpType.mult)
                nc.vector.tensor_add(out=e0[:], in0=e0[:], in1=ag[:])
                nc.sync.dma_start(
                    out=shard_rows_ap(eshard, i0, nt),
                    in_=e0[:].rearrange("p (t d) -> p t d", d=D))
            nc.gpsimd.collective_compute(
                "AllGather", mybir.AluOpType.bypass, replica_groups=rg,
                ins=[eshard[:].opt()], outs=[ego_full[0][:].opt()])

            # ---- layers ----
            for k in range(L):
                last = k == L - 1
                for w in range(n_win):
                    i0 = w * GT
                    nt = win_sizes[w]
                    sw = int(S_w[w])
                    so = int(s_woff[w])
                    idxw = io.tile([P, sw * 8], i16, tag="idxw")
                    nc.sync.dma_start(out=idxw[:],
                                      in_=idxg_t[:, so * 8:(so + sw) * 8])
                    valw = io.tile([P, sw], f32, tag="valw")
                    nc.sync.dma_start(out=valw[:], in_=valg_t[:, so:so + sw])
                    nzw = io.tile([P, nt * D], f32, tag="nzw")
                    nc.sync.dma_start(
                        out=nzw[:], in_=noisec_t[k, :, i0 * D:(i0 + nt) * D])

                    G = gat.tile([P, sw * D], f32, tag="G")
                    o = 0
                    for b in range(NB):
                        cb = int(caps[w][b])
                        if cb == 0:
                            continue
                        ni = nt * cb * P
                        nc.gpsimd.dma_gather(
                            out_ap=G[:, o * D:(o + nt * cb) * D].rearrange(
                                "p (s d) -> p s d", d=D),
                            in_ap=ego_full[k][b * BANK:
                                              min((b + 1) * BANK, NP), :],
                            idxs_ap=idxw[:, o * 8:(o + nt * cb) * 8],
                            num_idxs=ni, num_idxs_reg=ni, elem_size=D,
                            single_packet=False)
                        o += nt * cb
                    # multiply all slots by vals
                    nc.vector.tensor_tensor(
                        out=G[:].rearrange("p (s d) -> p s d", d=D),
                        in0=G[:].rearrange("p (s d) -> p s d", d=D),
                        in1=valw[:].unsqueeze(2).to_broadcast([P, sw, D]),
                        op=mybir.AluOpType.mult)
                    # per-bank reduction into pre
                    pre = work.tile([P, nt * D], f32, tag="pre")
                    tmp = work.tile([P, nt * D], f32, tag="tmp")
                    o = 0
                    first = True
                    for b in range(NB):
                        cb = int(caps[w][b])
                        if cb == 0:
                            continue
                        blk = G[:, o * D:(o + nt * cb) * D].rearrange(
                            "p (i c d) -> p i d c", c=cb, d=D)
                        dst = pre if first else tmp
                        nc.vector.tensor_reduce(
                            out=dst[:], in_=blk, op=mybir.AluOpType.add,
                            axis=mybir.AxisListType.X)
                        if not first:
                            nc.vector.tensor_add(out=pre[:], in0=pre[:],
                                                 in1=tmp[:])
                        first = False
                        o += nt * cb

                    # epilogue
                    sqv = work.tile([P, nt * D], f32, tag="sqv")
                    nc.scalar.activation(sqv[:], nzw[:],
                                         mybir.ActivationFunctionType.Square)
                    ssum = work.tile([P, nt], f32, tag="ssum")
                    nc.vector.tensor_reduce(
                        out=ssum[:],
                        in_=sqv[:].rearrange("p (t d) -> p t d", d=D),
                        op=mybir.AluOpType.add, axis=mybir.AxisListType.X)
                    nrm = work.tile([P, nt], f32, tag="nrm")
                    nc.scalar.activation(nrm[:], ssum[:],
                                         mybir.ActivationFunctionType.Sqrt,
                                         bias=biast[:], scale=inv_eps_sq)
                    rr = work.tile([P, nt], f32, tag="rr")
                    nc.vector.reciprocal(rr[:], nrm[:])
                    snk = work.tile([P, nt * D], f32, tag="snk")
                    nc.vector.tensor_tensor(
                        out=snk[:].rearrange("p (t d) -> p t d", d=D),
                        in0=nzw[:].rearrange("p (t d) -> p t d", d=D),
                        in1=rr[:].unsqueeze(2).to_broadcast([P, nt, D]),
                        op=mybir.AluOpType.mult)
                    sgn = work.tile([P, nt * D], f32, tag="sgn")
                    nc.scalar.activation(sgn[:], pre[:],
                                         mybir.ActivationFunctionType.Sign)
                    nc.vector.tensor_tensor(out=snk[:], in0=sgn[:], in1=snk[:],
                                            op=mybir.AluOpType.mult)
                    stage = work.tile([P, nt * D], f32, tag="stage")
                    nc.vector.tensor_add(out=stage[:], in0=pre[:], in1=snk[:])
                    # acc update (DRAM)
                    accw = work.tile([P, nt * D], f32, tag="accw")
                    if k > 0:
                        nc.sync.dma_start(out=accw[:],
                                          in_=shard_rows_ap(accd, i0, nt))
                        nc.vector.tensor_add(out=accw[:], in0=accw[:],
                                             in1=stage[:])
                    else:
                        nc.vector.tensor_copy(out=accw[:], in_=stage[:])
                    if last:
                        nc.vector.tensor_scalar_mul(stage[:], accw[:], 1.0 / L)
                        nc.sync.dma_start(
                            out=shard_rows_ap(out_t, i0, nt),
                            in_=stage[:].rearrange("p (t d) -> p t d", d=D))
                    else:
                        nc.sync.dma_start(
                            out=shard_rows_ap(accd, i0, nt),
                            in_=accw[:].rearrange("p (t d) -> p t d", d=D))
                        nc.sync.dma_start(
                            out=shard_rows_ap(eshard, i0, nt),
                            in_=stage[:].rearrange("p (t d) -> p t d", d=D))
                if not last:
                    nc.gpsimd.collective_compute(
                        "AllGather", mybir.AluOpType.bypass, replica_groups=rg,
                        ins=[eshard[:].opt()],
                        outs=[ego_full[k + 1][:].opt()])
    nc.compile()
    return nc


_CACHE = {}


def _prepare(cfg, user_emb, item_emb, author_emb, item2author, row, col, vals,
             noise):
    key = hashlib.sha1(
        row.tobytes() + col.tobytes() + item2author.tobytes()
        + np.int64(cfg.N).tobytes() + np.int64(cfg.GT).tobytes()).hexdigest()
    if key not in _CACHE:
        meta = preprocess(cfg, row, col, vals, item2author)
        nc = build_program(cfg, meta)
        _CACHE[key] = (meta, nc)
    return _CACHE[key]


def run(cfg, inputs, trace=False):
    from concourse import bass_utils

    user_emb = np.asarray(inputs["user_emb"], np.float32)
    item_emb = np.asarray(inputs["item_emb"], np.float32)
    author_emb = np.asarray(inputs["author_emb"], np.float32)
    item2author = np.asarray(inputs["item2author"], np.int32)
    row = np.asarray(inputs["row"], np.int32)
    col = np.asarray(inputs["col"], np.int32)
    vals = np.asarray(inputs["vals"], np.float32)
    noise = np.asarray(inputs["noise"], np.float32)

    meta, nc = _prepare(cfg, user_emb, item_emb, author_emb, item2author,
                        row, col, vals, noise)
    in_maps = build_in_maps(cfg, meta, user_emb, item_emb, author_emb, noise)
    res = bass_utils.run_bass_kernel_spmd(
        nc, in_maps, core_ids=list(range(cfg.n_cores)), trace=trace)
    shards = np.concatenate([res.results[c]["outshard"]
                             for c in range(cfg.n_cores)], axis=0)
    final = shards[meta["new_row"][:cfg.N].astype(np.int64)]
    return (np.ascontiguousarray(final[:cfg.num_users]),
            np.ascontiguousarray(final[cfg.num_users:])), res


def kernel(user_emb, item_emb, author_emb, item2author, row, col, vals, noise):
    (u, i), _ = run(FULL, dict(user_emb=user_emb, item_emb=item_emb,
                               author_emb=author_emb, item2author=item2author,
                               row=row, col=col, vals=vals, noise=noise))
    return u, i
